# revision 10
# baseline (speedup 1.0000x reference)
"""Trainium2 Bass kernel for the CRW palindrome-walk contrastive loss.

Reference computation (per batch b):
  f = L2-normalize(feats, axis=C)
  A_t = f_t^T f_{t+1}                      [N,N], t = 0..T-2
  R_t = rowsoftmax(A_t / tau)              (right edges)
  L_t = rowsoftmax(A_t^T / tau)            (left edges)
  for i in 1..T-2:
    path_i = R_0 R_1 .. R_i L_i L_{i-1} .. L_0
    loss_i = -mean_n log_softmax(log(path_i + EPS))[n, n]
  loss = mean_i loss_i

Device algorithm (per core, B/8 = 2 batches), normalized-weight form:
  * Track Qhat_i ~ (R_0..R_i)^T and Shat_i ~ L_i..L_0 column/row-scaled:
      Qhat_i = Rhat_i^T Qhat_{i-1} / ALPHA      (Rhat = ALPHA * rowsoftmax(E))
      Shat_i = (Ehat_i^T Shat_{i-1}) / colsum(Ehat_i)   (self-normalizing)
    diag(path_i) = colsum_k(Qhat_i * Shat_i) up to exactly-tracked scales.
  * Qhat_0 = DELTA*exp(A_0^T/tau) keeps its column scale folded into the
    loss via ledger_j = ln(colsum_j Qhat_0) (columns are preserved by both
    recurrences, and the ledger over the *quantized* seed is exact).
  * Shat_0 = ALPHA * rowsoftmax(exp(A_0^T/tau)).
  * FP8 mode: Ehat in e5m2 (range), Rhat/Qhat/Shat in e4m3 (precision),
    walk matmuls in DoubleRow mode (2 fp8 MACs/PE/cycle).  colsum(Ehat)
    rides as tiny ones-moving matmuls accumulated in PSUM.
  * Rows of path_i sum to 1, so log_softmax == log(diag) exactly up to
    ~1e-17; each core returns [1, N] summed log-diagonals (plus ledger
    terms); the host sums across cores in float64 and adds constants.
"""

import math
import threading

import numpy as np

import concourse.bass as bass  # noqa: F401  (engine types come via nc)
import concourse.tile as tile
import concourse.mybir as mybir
from concourse import bacc
from concourse.bass_utils import run_bass_kernel_spmd

B, C, T, N = 16, 128, 8, 1024
NCORES = 8
BPC = B // NCORES          # batches per core
TEMP = 0.07

FP8 = False                # False -> bf16 walk (safe), True -> fp8 DoubleRow

F32 = mybir.dt.float32
F32R = mybir.dt.float32r
BF16 = mybir.dt.bfloat16
E4 = mybir.dt.float8e4
E5 = mybir.dt.float8e5
EXP = mybir.ActivationFunctionType.Exp
LN = mybir.ActivationFunctionType.Ln
COPY = mybir.ActivationFunctionType.Copy
MUL = mybir.AluOpType.mult
DIV = mybir.AluOpType.divide

if FP8:
    W_S = E5               # Ehat (S-chain weights + t0 seed): needs range
    W_Q = E4               # Rhat (Q-chain weights): needs mantissa
    MV = E4                # Qhat/Shat moving states
    ALPHA = 128.0          # state scale
    DELTA = 0.5            # Ehat/seed scale
    KS = 2                 # contraction blocks per matmul (DoubleRow)
    PERF = mybir.MatmulPerfMode.DoubleRow
else:
    W_S = W_Q = MV = BF16
    ALPHA = 1.0
    DELTA = 1.0
    KS = 1
    PERF = None


def _r(ap):
    """View an fp32 AP as float32r for full-rate PE matmuls."""
    return ap.bitcast(F32R)


def build(n=N, t_len=T, bpc=BPC, n_cores=NCORES):
    NB = n // 128            # partition blocks per matrix dim
    CHN = 512 if n >= 512 else n
    NCH = n // CHN           # psum-width chunks per matrix dim
    NKS = NB // KS           # matmuls per contraction
    n_steps = t_len - 2      # walk steps i = 1..n_steps

    nc = bacc.Bacc("TRN2", target_bir_lowering=False, debug=False,
                   num_devices=n_cores)
    # Consts for activation bias lowering (must be pre-registered).
    for cval in {0.0, math.log(DELTA)}:
        ct = nc.alloc_sbuf_tensor(f"const-{cval}", [128, 1], F32)
        nc.gpsimd.memset(ct.ap(), cval)
        nc.const_aps.aps[(F32, cval)] = ct.ap()
    nc.all_engine_barrier()
    feats_d = nc.dram_tensor("feats", [bpc, C, t_len, n], F32,
                             kind="ExternalInput")
    out_d = nc.dram_tensor("out", [1, n], F32, kind="ExternalOutput")
    feats_ap = feats_d.ap()
    out_ap = out_d.ap()

    with tile.TileContext(nc) as tc:
        with (
            tc.tile_pool(name="const", bufs=1) as const_pool,
            tc.tile_pool(name="fraw", bufs=3) as fraw_pool,   # 3x4K
            tc.tile_pool(name="fh", bufs=3) as fh_pool,       # 3x2K
            tc.tile_pool(name="e", bufs=2) as e_pool,         # Ehat
            tc.tile_pool(name="r", bufs=2) as r_pool,         # Rhat
            tc.tile_pool(name="q", bufs=2) as q_pool,         # Qhat
            tc.tile_pool(name="s", bufs=2) as s_pool,         # Shat
            tc.tile_pool(name="d", bufs=2) as d_pool,         # Q*S products
            tc.tile_pool(name="cs", bufs=2) as cs_pool,       # norm recip
            tc.tile_pool(name="stat", bufs=4) as stat_pool,   # rowsums
            tc.tile_pool(name="lg", bufs=2) as lg_pool,       # [1,CHN]
            tc.tile_pool(name="acc", bufs=1) as acc_pool,     # [1,n]
            # PSUM: 2+2+2+1+1 = 8 banks
            tc.tile_pool(name="aps", bufs=2, space="PSUM") as aps,
            tc.tile_pool(name="qps", bufs=2, space="PSUM") as qps,
            tc.tile_pool(name="sps", bufs=2, space="PSUM") as sps,
            tc.tile_pool(name="cps", bufs=1, space="PSUM") as cps,
            tc.tile_pool(name="dps", bufs=1, space="PSUM") as dps,
        ):
            ones_raw = const_pool.tile([128, 128], F32, tag="ones_raw")
            nc.vector.memset(ones_raw[:], 1.0)
            ones = const_pool.tile([128, 128], F32R, tag="ones")
            nc.scalar.copy(ones[:], ones_raw[:])
            ones_w = const_pool.tile([128, 1], W_S, tag="ones_w")
            nc.vector.memset(ones_w[:], 1.0)
            ones_d = const_pool.tile([128, 1], BF16, tag="ones_d")
            nc.vector.memset(ones_d[:], 1.0)
            ones_mv = const_pool.tile([128, KS, 2], MV, tag="ones_mv")
            nc.vector.memset(ones_mv[:], 1.0)
            loss_acc = acc_pool.tile([1, n], F32, tag="acc")
            nc.vector.memset(loss_acc[:], 0.0)

            def chs(ch):
                return slice(ch * CHN, (ch + 1) * CHN)

            def load_slice(b, t):
                """DMA feats[b,:,t,:] then L2-normalize columns -> bf16."""
                f = fraw_pool.tile([128, n], F32, tag="fraw")
                nc.sync.dma_start(f[:], feats_ap[b, :, t, :])
                fh = fh_pool.tile([128, n], BF16, tag="fh")
                for ch in range(NCH):
                    sq = cs_pool.tile([128, CHN], F32R, tag="sq")
                    nc.scalar.square(sq[:], f[:, chs(ch)])
                    nps = aps.tile([128, CHN], F32, tag="aps")
                    nc.tensor.matmul(nps[:], _r(ones[:]), sq[:],
                                     start=True, stop=True)
                    nrm = cs_pool.tile([128, CHN], F32, tag="cs")
                    nc.scalar.sqrt(nrm[:], nps[:])
                    nc.vector.tensor_scalar_add(nrm[:], nrm[:], 1e-12)
                    nc.vector.reciprocal(nrm[:], nrm[:])
                    nc.vector.tensor_mul(fh[:, chs(ch)], f[:, chs(ch)], nrm[:])
                return fh

            def affinity_exp(fL, fR, out_dt, tag):
                """E-hat tile [128, NB, n] = out_dt(DELTA * exp(A/tau)) with
                A[k, j] = sum_c fL[c, k] fR[c, j]; returns (ehat, rs[128,NB])."""
                ehat = (e_pool if tag == "e" else q_pool).tile(
                    [128, NB, n], out_dt, tag=tag)
                rsh0 = stat_pool.tile([128, NB], F32, tag="rs0")
                rsh1 = stat_pool.tile([128, NB], F32, tag="rs1")
                rsh = [rsh0, rsh1]
                for nb in range(NB):
                    nbs = slice(nb * 128, (nb + 1) * 128)
                    for h in range(NCH):
                        a_ps = aps.tile([128, CHN], F32, tag="aps")
                        nc.tensor.matmul(a_ps[:], fL[:, nbs], fR[:, chs(h)],
                                         start=True, stop=True)
                        nc.scalar.activation(
                            ehat[:, nb, chs(h)], a_ps[:], EXP,
                            scale=1.0 / TEMP, bias=math.log(DELTA),
                            accum_out=rsh[h][:, nb:nb + 1])
                rs = stat_pool.tile([128, NB], F32, tag="rst")
                nc.vector.tensor_add(rs[:], rsh[0][:], rsh[1][:])
                return ehat, rs

            def make_rhat(ehat, rs):
                """Rhat = (ALPHA/DELTA) * Ehat / rs  (row-softmax weights)."""
                rsi = stat_pool.tile([128, NB], F32, tag="rsi")
                nc.vector.reciprocal(rsi[:], rs[:])
                rhat = r_pool.tile([128, NB, n], W_Q, tag="r")
                for nb in range(NB):
                    nc.vector.tensor_scalar(
                        out=rhat[:, nb, :], in0=ehat[:, nb, :],
                        scalar1=rsi[:, nb:nb + 1], scalar2=ALPHA / DELTA,
                        op0=MUL, op1=MUL)
                return rhat

            def colsum_ledger(qhat0, weight):
                """loss_acc[j] += weight * ln(colsum_j Qhat_0)."""
                for h in range(NCH):
                    d_ps = dps.tile([1, CHN], F32, tag="dps")
                    for kb in range(NB):
                        nc.tensor.matmul(d_ps[:], ones_w[:],
                                         qhat0[:, kb, chs(h)],
                                         start=(kb == 0), stop=(kb == NB - 1))
                    lgc = lg_pool.tile([1, CHN], F32, tag="lg")
                    nc.scalar.activation(lgc[:], d_ps[:], LN)
                    nc.vector.tensor_scalar(
                        out=lgc[:], in0=lgc[:], scalar1=weight,
                        scalar2=None, op0=MUL)
                    nc.vector.tensor_add(loss_acc[:, chs(h)],
                                         loss_acc[:, chs(h)], lgc[:])

            def walk_step(ehat_t, rhat_t, qcur, scur):
                """One palindrome step; returns (qnew, snew)."""
                # Q chain: Qhat_new = Rhat^T Qhat / ALPHA   (ACT evacuation)
                qnew = q_pool.tile([128, NB, n], MV, tag="q")
                for ch in range(NCH):
                    for mb in range(NB):
                        mbs = slice(mb * 128, (mb + 1) * 128)
                        qp = qps.tile([128, CHN], F32, tag="qps")
                        for i, p in enumerate(range(0, NB, KS)):
                            nc.tensor.matmul(qp[:], rhat_t[:, p:p + KS, mbs],
                                             qcur[:, p:p + KS, chs(ch)],
                                             start=(i == 0),
                                             stop=(i == NKS - 1),
                                             perf_mode=PERF)
                        nc.scalar.activation(qnew[:, mb, chs(ch)], qp[:],
                                             COPY, scale=1.0 / ALPHA)
                return qnew

            def walk_step_s(ehat_t, scur):
                # colsum(Ehat) per output block, in-psum: c[m] on partition m
                c_ps = cps.tile([128, 2 * NB], F32, tag="cps")
                for mb in range(NB):
                    mbs = slice(mb * 128, (mb + 1) * 128)
                    for i, p in enumerate(range(0, NB, KS)):
                        nc.tensor.matmul(c_ps[:, 2 * mb:2 * mb + 2],
                                         ehat_t[:, p:p + KS, mbs],
                                         ones_mv[:],
                                         start=(i == 0), stop=(i == NKS - 1),
                                         perf_mode=PERF)
                cinv = cs_pool.tile([128, 2 * NB], F32, tag="cinv")
                nc.vector.reciprocal(cinv[:], c_ps[:])
                # S chain: Shat_new = Ehat^T Shat / colsum  (DVE evacuation)
                snew = s_pool.tile([128, NB, n], MV, tag="s")
                for ch in range(NCH):
                    for mb in range(NB):
                        mbs = slice(mb * 128, (mb + 1) * 128)
                        sp = sps.tile([128, CHN], F32, tag="sps")
                        for i, p in enumerate(range(0, NB, KS)):
                            nc.tensor.matmul(sp[:], ehat_t[:, p:p + KS, mbs],
                                             scur[:, p:p + KS, chs(ch)],
                                             start=(i == 0),
                                             stop=(i == NKS - 1),
                                             perf_mode=PERF)
                        nc.vector.tensor_scalar(
                            out=snew[:, mb, chs(ch)], in0=sp[:],
                            scalar1=cinv[:, 2 * mb:2 * mb + 1], scalar2=None,
                            op0=MUL)
                return snew

            def diag_mul(qnew, snew):
                """dt[k, j] = Qhat[k, j] * Shat[k, j] (bf16)."""
                dt = d_pool.tile([128, NB, n], BF16, tag="d")
                for kb in range(NB):
                    nc.vector.tensor_mul(dt[:, kb, :], qnew[:, kb, :],
                                         snew[:, kb, :])
                return dt

            def diag_reduce(dt):
                """loss_acc[j] += ln(colsum_k dt)."""
                for h in range(NCH):
                    d_ps = dps.tile([1, CHN], F32, tag="dps")
                    for kb in range(NB):
                        nc.tensor.matmul(d_ps[:], ones_d[:],
                                         dt[:, kb, chs(h)],
                                         start=(kb == 0), stop=(kb == NB - 1))
                    lgc = lg_pool.tile([1, CHN], F32, tag="lg")
                    nc.scalar.activation(lgc[:], d_ps[:], LN)
                    nc.vector.tensor_add(loss_acc[:, chs(h)],
                                         loss_acc[:, chs(h)], lgc[:])

            for b in range(bpc):
                fh0 = load_slice(b, 0)
                fh1 = load_slice(b, 1)
                # t0 seed from E' = DELTA * exp(A_0^T / tau)
                qhat0, rs0 = affinity_exp(fh1, fh0, W_S, tag="q")
                rs0i = stat_pool.tile([128, NB], F32, tag="rsi")
                nc.vector.reciprocal(rs0i[:], rs0[:])
                scur = s_pool.tile([128, NB, n], MV, tag="s")
                for nb in range(NB):
                    nc.vector.tensor_scalar(
                        out=scur[:, nb, :], in0=qhat0[:, nb, :],
                        scalar1=rs0i[:, nb:nb + 1], scalar2=ALPHA / DELTA,
                        op0=MUL, op1=MUL)
                colsum_ledger(qhat0, -float(n_steps))
                qcur = qhat0
                fh_prev = fh1
                fh_next = load_slice(b, 2)
                ehat, rs = affinity_exp(fh_prev, fh_next, W_S, tag="e")
                rhat = make_rhat(ehat, rs)
                pending_dt = None
                for t in range(1, n_steps + 1):
                    qnew = walk_step(ehat, rhat, qcur, scur)
                    if pending_dt is not None:
                        diag_reduce(pending_dt)
                    if t < n_steps:
                        fh_prev, fh_next = fh_next, load_slice(b, t + 2)
                        ehat_n, rs_n = affinity_exp(fh_prev, fh_next, W_S,
                                                    tag="e")
                    snew = walk_step_s(ehat, scur)
                    pending_dt = diag_mul(qnew, snew)
                    if t < n_steps:
                        rhat = make_rhat(ehat_n, rs_n)
                        ehat = ehat_n
                    qcur, scur = qnew, snew
                diag_reduce(pending_dt)

            nc.sync.dma_start(out_ap[:, :], loss_acc[:])

    nc.compile()
    return nc


_build_lock = threading.Lock()
_built_nc = None


def _get_nc():
    global _built_nc
    with _build_lock:
        if _built_nc is None:
            _built_nc = build()
    return _built_nc


LAST_RESULT = None  # BassKernelResults of the most recent run (for profiling)


def kernel(feats: np.ndarray) -> np.ndarray:
    global LAST_RESULT
    feats = np.ascontiguousarray(np.asarray(feats), dtype=np.float32)
    assert feats.shape == (B, C, T, N), feats.shape
    nc = _get_nc()
    in_maps = [
        {"feats": np.ascontiguousarray(feats[c * BPC:(c + 1) * BPC])}
        for c in range(NCORES)
    ]
    res = run_bass_kernel_spmd(nc, in_maps, core_ids=list(range(NCORES)))
    LAST_RESULT = res
    total = 0.0
    for r in res.results:
        total += r["out"].astype(np.float64).sum()
    n_walks = T - 2  # i = 1..T-2 inclusive
    loss = -total / (n_walks * B * N) + math.log(ALPHA)
    return np.float32(loss)
